# revision 44
# baseline (speedup 1.0000x reference)
"""GQA causal attention with RoPE, distributed over 8 trn2 NeuronCores.

Sharding: 4-way data parallel over batch x 2-way tensor parallel over heads.
Core c = 2*b + t handles batch b with query heads [t*8, (t+1)*8) and KV heads
[t*2, (t+1)*2).  Each core computes a row-sharded out_proj partial; the pair
partials are summed on the host during unsharding.

On-chip algorithm (per core, bf16 matmuls / fp32 softmax):
  Phase A (projections+RoPE), software-pipelined three deep:
    proj(m) on PE  ||  drain(m-1) PSUM->SBUF bf16 on ScalarE  ||
    rope(m-1) on DVE (bf16, 2x mode)  ||  transpose-pack(m-2) on PE.
    The ScalarE drain frees the PSUM accumulator ~1.5us after the last
    matmul so proj(m+1) never stalls on the RoPE chain.
    Input DMAs are split into ~128KB column pieces and issued from three
    sequencers (SP / ScalarE / DVE) in consumption order so the HBM ramp
    tracks the PE's k-sweep.
  Phase B (causal attention), scoresT[k_tok, q_tok] so probs feed the AV
    matmul untransposed.  exp on ScalarE, diag-mask + early column-sum
    accumulation on DVE, late (short) column-sums on GpSimd, denominator
    broadcast via ones-matmul on PE, per-head finalize deferred into the
    next head's pipeline.
  Phase C out-projection: PE-saturated; PSUM->SBUF copies on ScalarE, the
    last chunk split into 512-col pieces alternating ScalarE/DVE so the
    final HBM store starts as early as possible.
"""

import math
import sys

sys.path.insert(0, "/opt/trn_rl_repo")

import ml_dtypes
import numpy as np

import concourse.bacc as bacc
import concourse.bass_isa as bass_isa
import concourse.mybir as mybir
import concourse.tile as tile
from concourse.bass import _add_dep_helper
from concourse.bass_utils import run_bass_kernel_spmd
from concourse.masks import make_identity, make_lower_triangular

B, S, HID = 4, 1024, 2048
H, KV, D = 16, 4, 128
P = 128
TP = 2                  # tensor-parallel ways (head split)
HL = H // TP            # 8 query heads per core
KVL = KV // TP          # 2 kv heads per core
QD = HL * D             # 1024
KD = KVL * D            # 256
SC = S // P             # 8 token chunks
KC = HID // P           # 16 hidden chunks
NCORES = 8
BF = mybir.dt.bfloat16
F32 = mybir.dt.float32
Exp = mybir.ActivationFunctionType.Exp
QKVD = QD + 2 * KD      # 1536 = q 1024 | k 256 | v 256

_NC_CACHE = {}


def _ensure_ntff_hook():
    """The agent image's antenv lacks axon_hooks, so bass_utils' trace=True
    path can't find the NTFF profile hook trn_boot would have registered.
    Recreate the module and register the ctypes-based hook ourselves."""
    try:
        from antenv.axon_hooks import get_axon_ntff_profile_hook  # noqa: F401
        return
    except ImportError:
        pass
    import types

    import antenv

    mod = types.ModuleType("antenv.axon_hooks")
    _state = {"hook": None}
    mod.set_axon_ntff_profile_hook = lambda h: _state.__setitem__("hook", h)
    mod.get_axon_ntff_profile_hook = lambda: _state["hook"]
    sys.modules["antenv.axon_hooks"] = mod
    antenv.axon_hooks = mod
    try:
        from trn_agent_boot.trn_boot import _ntff_profile_via_ctypes

        hook = _ntff_profile_via_ctypes("/opt/axon/libaxon_pjrt.so")
        if hook is not None:
            mod.set_axon_ntff_profile_hook(hook)
    except Exception as e:  # pragma: no cover
        print(f"NTFF hook registration failed: {e}", file=sys.stderr)


def _pieces(start, end, step=512):
    """Split [start, end) into spans of at most `step`, aligned so no span
    crosses a `step` boundary (PSUM: one bank per matmul)."""
    out = []
    a = start
    while a < end:
        b = min((a // step + 1) * step, end)
        out.append((a, b))
        a = b
    return out


def build_nc():
    nc = bacc.Bacc("TRN2", target_bir_lowering=False, debug=False,
                   num_devices=NCORES)

    xT = nc.declare_dram_parameter("xT", [HID, S], BF, isOutput=False)
    wqkv = nc.declare_dram_parameter("wqkv", [HID, QKVD], BF, isOutput=False)
    wo = nc.declare_dram_parameter("wo", [QD, HID], BF, isOutput=False)
    cos_t = nc.declare_dram_parameter("cos_t", [S, D], BF, isOutput=False)
    sin_t = nc.declare_dram_parameter("sin_t", [S, D], BF, isOutput=False)
    out = nc.declare_dram_parameter("out", [S, HID], BF, isOutput=True)

    HALF = D // 2

    with tile.TileContext(nc) as tc:
        with (
            tc.tile_pool(name="consts", bufs=1) as cpool,
            tc.tile_pool(name="wpool", bufs=1) as wpool,
            tc.tile_pool(name="qkvpool", bufs=1) as qkvpool,
        ):
            ident = cpool.tile([P, P], BF)
            make_identity(nc, ident[:, :])
            # additive causal mask: -BIG strictly below the diagonal of the
            # scoresT diag block, accumulated into the scores on the PE
            # (ident.T @ ltmask) so exp gives exact zeros there
            ltmask = cpool.tile([P, P], BF)
            make_lower_triangular(nc, ltmask[:, :], val=-30000.0, diag=False)
            ones_mat = cpool.tile([P, P], BF)
            nc.vector.memset(ones_mat[:, :], 1.0)

            sb_wo = wpool.tile([P, HL, HID], BF)

            sb_qT = qkvpool.tile([P, HL, S], BF)      # feature-major q
            sb_kT = qkvpool.tile([P, KVL, S], BF)     # feature-major k
            sb_v = qkvpool.tile([P, SC, KD], BF)      # token-major v
            sb_attnT = qkvpool.tile([P, HL, S], BF)   # feature-major attn out

            # ---------------- Phase A: projections + RoPE -----------------
            with (
                tc.tile_pool(name="proj", bufs=1) as projpool,
                tc.tile_pool(name="drain", bufs=3) as drpool,
                tc.tile_pool(name="rope", bufs=3) as ropepool,
                tc.tile_pool(name="ps_q", bufs=2, space="PSUM") as ps_q,
            ):
                sb_xT = projpool.tile([P, KC, S], BF)
                sb_wqkv = projpool.tile([P, KC, QKVD], BF)
                xT_r = xT.rearrange("(c p) s -> p c s", p=P)
                wqkv_r = wqkv.rearrange("(c p) n -> p c n", p=P)

                # Input DMA plan.  Each dma_start is striped across all 16
                # DMA engines, so landing latency ~ size and issue density;
                # sequencers issue one dma_start per ~0.6us.  Ship each
                # hidden-chunk as TWO row-half DMAs (full 2-3KB descriptor
                # lines, half the landing latency), issued from SP and
                # ScalarE in lockstep with the k-sweep's consumption order.
                sb_ck = projpool.tile([P, SC, D], BF)
                sb_sk = projpool.tile([P, SC, D], BF)
                # per k the m0/m1 sweep needs x-p0 (the lhsT — hard gate)
                # first, then w n0/n1/n2; x-p1 only from m=4 (~t=60us)
                for k in range(KC):
                    nc.sync.dma_start(out=sb_xT[:, k, 0:512],
                                      in_=xT_r[:, k, 0:512])
                    nc.sync.dma_start(out=sb_wqkv[:, k, 0:512],
                                      in_=wqkv_r[:, k, 0:512])
                    nc.scalar.dma_start(out=sb_wqkv[:, k, 512:1024],
                                        in_=wqkv_r[:, k, 512:1024])
                    nc.scalar.dma_start(out=sb_wqkv[:, k, 1024:QKVD],
                                        in_=wqkv_r[:, k, 1024:QKVD])
                    nc.gpsimd.dma_start(out=sb_xT[:, k, 512:S],
                                        in_=xT_r[:, k, 512:S])
                # rope(m0) needs the tables at ~t=50us
                nc.gpsimd.dma_start(
                    out=sb_ck[:, :, :],
                    in_=cos_t.rearrange("(m p) d -> p m d", p=P),
                )
                nc.gpsimd.dma_start(
                    out=sb_sk[:, :, :],
                    in_=sin_t.rearrange("(m p) d -> p m d", p=P),
                )
                # wo is only needed in phase C: delay its (4 MB) load until
                # the input streaming has drained (dep attached below)
                wo_dma = nc.scalar.dma_start(
                    out=sb_wo[:, :, :],
                    in_=wo.rearrange("(c p) n -> p c n", p=P),
                )

                def proj_mms(pqkv, m, k):
                    st, sp = (k == 0), (k == KC - 1)
                    lhsT = sb_xT[:, k, m * P:(m + 1) * P]
                    mm = None
                    for n in range(QKVD // 512):
                        mm = nc.tensor.matmul(
                            pqkv[:, n * 512:(n + 1) * 512], lhsT,
                            sb_wqkv[:, k, n * 512:(n + 1) * 512],
                            start=st, stop=sp)
                    return mm

                def drain(pqkv, m, dr, v_on_vector=False):
                    """Free the PSUM accumulator fast: ScalarE copies the
                    q|k span to a bf16 staging tile and the v span straight
                    into sb_v."""
                    nc.scalar.copy(dr[:, :], pqkv[:, 0:QD + KD])
                    if v_on_vector:
                        nc.vector.tensor_copy(sb_v[:, m, :],
                                              pqkv[:, QD + KD:QKVD])
                    else:
                        nc.scalar.copy(sb_v[:, m, :], pqkv[:, QD + KD:QKVD])

                def rope_block(dr, lo, nh, m, tag):
                    """RoPE `nh` consecutive heads of the drained bf16 tile
                    (cols [lo, lo+nh*D)) in batched bf16 DVE ops (2x mode).
                    sin table is pre-negated on its first half."""
                    t1 = ropepool.tile([P, nh, D], BF, tag=f"t1_{tag}")
                    ro = ropepool.tile([P, nh * D], BF, tag=f"ro_{tag}")
                    src = dr[:, lo:lo + nh * D].rearrange(
                        "p (h d) -> p h d", h=nh)
                    sin_lo = sb_sk[:, m:m + 1, 0:HALF].broadcast_to(
                        [P, nh, HALF])
                    sin_hi = sb_sk[:, m:m + 1, HALF:D].broadcast_to(
                        [P, nh, HALF])
                    cos_b = sb_ck[:, m:m + 1, :].broadcast_to([P, nh, D])
                    nc.vector.tensor_mul(t1[:, :, 0:HALF], src[:, :, HALF:D],
                                         sin_lo)
                    nc.vector.tensor_mul(t1[:, :, HALF:D], src[:, :, 0:HALF],
                                         sin_hi)
                    ror = ro[:, :].rearrange("p (h d) -> p h d", h=nh)
                    nc.vector.tensor_mul(ror, src, cos_b)
                    nc.vector.tensor_add(ror, ror, t1[:, :, :])
                    return ro

                ropes = {}

                def emit_drain_rope(pqkv, m, last=False):
                    dr = drpool.tile([P, QD + KD], BF, tag="dr")
                    drain(pqkv, m, dr, v_on_vector=last)
                    k_ro = rope_block(dr, QD, KVL, m, "k")
                    q_ro = rope_block(dr, 0, HL, m, "q")
                    ropes[m] = (k_ro, q_ro)

                # Ramp: m=0, m=1 and m2's q-half interleaved per k-chunk so
                # each arriving chunk feeds ~3x the matmul work while HBM
                # streams in (the ramp is HBM-arrival-bound; more in-flight
                # rows mean less serial catch-up after it).  m2's q-half
                # borrows the PSUM banks the transpose pool uses later.
                pqkv0 = ps_q.tile([P, QKVD], F32, tag="pqkv")
                pqkv1 = ps_q.tile([P, QKVD], F32, tag="pqkv")
                with tc.tile_pool(name="ps_r", bufs=1, space="PSUM") as ps_r:
                    m2q = ps_r.tile([P, QD], F32)
                    for k in range(KC):
                        proj_mms(pqkv0, 0, k)
                        proj_mms(pqkv1, 1, k)
                        lhsT = sb_xT[:, k, 2 * P:3 * P]
                        for n in range(2):
                            nc.tensor.matmul(
                                m2q[:, n * 512:(n + 1) * 512], lhsT,
                                sb_wqkv[:, k, n * 512:(n + 1) * 512],
                                start=(k == 0), stop=(k == KC - 1))
                    emit_drain_rope(pqkv0, 0)
                    # m2's kv columns in a short re-sweep (ring slot freed
                    # by drain(m0))
                    pq2 = ps_q.tile([P, QKVD], F32, tag="pqkv")
                    for k in range(KC):
                        nc.tensor.matmul(pq2[:, QD:QKVD],
                                         sb_xT[:, k, 2 * P:3 * P],
                                         sb_wqkv[:, k, QD:QKVD],
                                         start=(k == 0), stop=(k == KC - 1))
                    emit_drain_rope(pqkv1, 1)
                    # drain m2 from its two accumulators
                    dr2 = drpool.tile([P, QD + KD], BF, tag="dr")
                    nc.scalar.copy(dr2[:, 0:QD], m2q[:, :])
                    nc.scalar.copy(dr2[:, QD:QD + KD], pq2[:, QD:QD + KD])
                    nc.scalar.copy(sb_v[:, 2, :], pq2[:, QD + KD:QKVD])
                    ropes[2] = (rope_block(dr2, QD, KVL, 2, "k"),
                                rope_block(dr2, 0, HL, 2, "q"))

                with tc.tile_pool(name="ps_t", bufs=2,
                                  space="PSUM") as ps_t:
                    def transpose_pack(ro, nh, dst, on_vector=False):
                        """PE-transpose nh [P, D] chunks of ro into one
                        packed PSUM tile, then one copy into dst."""
                        pt_full = ps_t.tile([P, 4 * P], BF, tag="pt")
                        pt = pt_full[:, 0:nh * P]
                        for i in range(nh):
                            nc.tensor.matmul(pt[:, i * P:(i + 1) * P],
                                             ro[:, i * D:(i + 1) * D],
                                             ident[:, :], is_transpose=True,
                                             start=(i == 0),
                                             stop=(i == nh - 1))
                        src = pt[:, :].rearrange("p (h t) -> p h t", h=nh)
                        if on_vector:
                            nc.vector.tensor_copy(dst, src)
                        else:
                            nc.scalar.copy(dst, src)

                    def emit_transposes(m, on_vector=False):
                        k_ro, q_ro = ropes.pop(m)
                        ms = slice(m * P, (m + 1) * P)
                        transpose_pack(k_ro, KVL, sb_kT[:, :, ms],
                                       on_vector=on_vector)
                        transpose_pack(q_ro[:, 0:4 * D], 4,
                                       sb_qT[:, 0:4, ms],
                                       on_vector=on_vector)
                        transpose_pack(q_ro[:, 4 * D:8 * D], 4,
                                       sb_qT[:, 4:8, ms],
                                       on_vector=on_vector)

                    # Steady pipeline:
                    #   proj(m) || drain/rope(m-1) || transp(m-3)
                    pqkvs = {}
                    for m in range(3, SC):
                        pqkv = ps_q.tile([P, QKVD], F32, tag="pqkv")
                        for k in range(KC):
                            mm = proj_mms(pqkv, m, k)
                            if m == 4 and k == 0:
                                _add_dep_helper(
                                    wo_dma.ins, mm.ins,
                                    reason="delay wo past input ramp")
                        if m > 3:
                            emit_drain_rope(pqkvs.pop(m - 1), m - 1)
                        emit_transposes(m - 3)
                        pqkvs[m] = pqkv
                    emit_drain_rope(pqkvs.pop(SC - 1), SC - 1, last=True)
                    emit_transposes(SC - 3)
                    emit_transposes(SC - 2)
                    emit_transposes(SC - 1, on_vector=True)

            # ---------------- Phase B: causal attention -------------------
            with (
                tc.tile_pool(name="attn", bufs=3) as attnpool,
                tc.tile_pool(name="norm", bufs=2) as normpool,
                tc.tile_pool(name="ps_sc", bufs=4, space="PSUM") as ps_sc,
                tc.tile_pool(name="ps_av", bufs=2, space="PSUM") as ps_av,
            ):
                def make_head(h, g, probsT, acc, accB, pav, rbc):
                    def av(ki):
                        st, sp = (ki == 0), (ki == SC - 1)
                        for (a, b) in _pieces(ki * P, S):
                            nc.tensor.matmul(pav[:, a:b],
                                             sb_v[:, ki, g * D:(g + 1) * D],
                                             probsT[:, ki, a:b],
                                             start=st, stop=sp)

                    def finalize():
                        av(SC - 1)
                        # ones-matmuls = column sums broadcast across all
                        # partitions.  The [P,512] psums ride the psc ring
                        # (same tag/shape); with a 4-deep ring their WAR
                        # deps clear without stalling the score matmuls.
                        # acc (ki0-3) and accB (ki4-7) are two independent
                        # DVE chains so the last head's tail is short; the
                        # second denominator piece sums both.
                        psbc = ps_sc.tile([P, 512], F32, tag="psc")
                        nc.tensor.matmul(psbc[:, :], ones_mat[:, :],
                                         acc[:, 0:512],
                                         start=True, stop=True)
                        nc.vector.reciprocal_approx_fast(
                            rbc[:, 0:512], psbc[:, :])
                        psbc2 = ps_sc.tile([P, 512], F32, tag="psc")
                        nc.tensor.matmul(psbc2[:, :], ones_mat[:, :],
                                         acc[:, 512:S],
                                         start=True, stop=False)
                        nc.tensor.matmul(psbc2[:, :], ones_mat[:, :],
                                         accB[:, :],
                                         start=False, stop=True)
                        nc.vector.reciprocal_approx_fast(
                            rbc[:, 512:S], psbc2[:, :])
                        nc.vector.tensor_mul(sb_attnT[:, h, :], pav[:, :],
                                             rbc[:, :])

                    return av, finalize

                pending = [None]
                for h in range(HL):
                    g = h // (HL // KVL)
                    probsT = attnpool.tile([P, SC, S], BF, tag="probsT")
                    acc = attnpool.tile([P, S], BF, tag="acc")
                    accB = attnpool.tile([P, 512], BF, tag="accB")
                    pav = ps_av.tile([P, S], F32, tag="pav")
                    rbc = normpool.tile([P, S], F32, tag="rbc")
                    av, finalize = make_head(h, g, probsT, acc, accB, pav,
                                             rbc)

                    for ki in range(SC):
                        q0 = ki * P
                        kslice = slice(q0, q0 + P)
                        for (a, b) in _pieces(q0, S):
                            psc = ps_sc.tile([P, 512], F32, tag="psc")
                            has_diag = a <= q0 < b
                            nc.tensor.matmul(psc[:, 0:b - a],
                                             sb_kT[:, g, kslice],
                                             sb_qT[:, h, a:b],
                                             start=True, stop=not has_diag)
                            if has_diag:
                                # accumulate the additive causal mask into
                                # the diag block (ident.T @ ltmask = ltmask)
                                nc.tensor.matmul(
                                    psc[:, q0 - a:q0 - a + P],
                                    ident[:, :], ltmask[:, :],
                                    start=False, stop=True)
                            nc.scalar.activation(probsT[:, ki, a:b],
                                                 psc[:, 0:b - a], Exp,
                                                 scale=float(1 / math.sqrt(D)))
                        # accumulate the column sums on DVE (bf16, 2x mode)
                        # in two independent chains: acc = ki0..3 (complete
                        # at ki3), accB = ki4..7 over cols [512:]
                        if ki == 1:
                            nc.vector.tensor_copy(acc[:, 0:P],
                                                  probsT[:, 0, 0:P])
                            nc.vector.tensor_add(acc[:, q0:],
                                                 probsT[:, 0, q0:],
                                                 probsT[:, 1, q0:])
                        elif ki in (2, 3):
                            nc.vector.tensor_add(acc[:, q0:], acc[:, q0:],
                                                 probsT[:, ki, q0:])
                        elif ki == 4:
                            nc.vector.tensor_copy(accB[:, :],
                                                  probsT[:, 4, 512:])
                        elif ki >= 5:
                            nc.vector.tensor_add(accB[:, q0 - 512:],
                                                 accB[:, q0 - 512:],
                                                 probsT[:, ki, q0:])
                        if ki >= 1:
                            av(ki - 1)
                        # previous head's finalize chain runs inside this
                        # head's compute instead of stalling the PE
                        if ki == 3 and pending[0] is not None:
                            pending[0]()
                            pending[0] = None

                    pending[0] = finalize
                pending[0]()

            # ---------------- Phase C: out projection ---------------------
            with (
                tc.tile_pool(name="ysb", bufs=2) as ypool,
                tc.tile_pool(name="ps_y", bufs=2, space="PSUM") as ps_y,
            ):
                for m in range(SC):
                    ms = slice(m * P, (m + 1) * P)
                    py = ps_y.tile([P, HID], F32, tag="py")
                    for k in range(HL):
                        st, sp = (k == 0), (k == HL - 1)
                        lhsT = sb_attnT[:, k, ms]
                        for n in range(HID // 512):
                            nc.tensor.matmul(py[:, n * 512:(n + 1) * 512],
                                             lhsT,
                                             sb_wo[:, k, n * 512:(n + 1) * 512],
                                             start=st, stop=sp)
                    ysb = ypool.tile([P, HID], BF, tag="ysb")
                    if m >= SC - 2:
                        # final chunks: copy + store per piece, engines
                        # alternating, so the last DMA starts early
                        np_ = 4
                        w = HID // np_
                        for n in range(np_):
                            ns = slice(n * w, (n + 1) * w)
                            if n % 2 == 0:
                                nc.scalar.copy(ysb[:, ns], py[:, ns])
                                nc.sync.dma_start(out=out[ms, ns],
                                                  in_=ysb[:, ns])
                            else:
                                nc.vector.tensor_copy(ysb[:, ns], py[:, ns])
                                nc.scalar.dma_start(out=out[ms, ns],
                                                    in_=ysb[:, ns])
                    else:
                        nc.scalar.copy(ysb[:, :], py[:, :])
                        nc.sync.dma_start(out=out[ms, :], in_=ysb[:, :])

    nc.compile()
    return nc


def _get_nc():
    if "nc" not in _NC_CACHE:
        _NC_CACHE["nc"] = build_nc()
    return _NC_CACHE["nc"]


def _make_in_maps(x, cos, sin, wq, wk, wv, wo):
    bf = ml_dtypes.bfloat16
    HALF = D // 2
    sin_rot = np.concatenate([-sin[:, :HALF], sin[:, HALF:]], axis=1)
    cos_t = cos.astype(bf)
    sin_t = sin_rot.astype(bf)
    in_maps = []
    for core in range(NCORES):
        b, t = divmod(core, TP)
        wqkv = np.concatenate([
            wq[:, t * QD:(t + 1) * QD],
            wk[:, t * KD:(t + 1) * KD],
            wv[:, t * KD:(t + 1) * KD],
        ], axis=1)
        in_maps.append({
            "xT": np.ascontiguousarray(x[b].T).astype(bf),
            "wqkv": np.ascontiguousarray(wqkv).astype(bf),
            "wo": np.ascontiguousarray(wo[t * QD:(t + 1) * QD, :]).astype(bf),
            "cos_t": cos_t, "sin_t": sin_t,
        })
    return in_maps


def run(inputs, trace=False):
    if trace:
        _ensure_ntff_hook()
    nc = _get_nc()
    in_maps = _make_in_maps(
        np.asarray(inputs["x"], np.float32),
        np.asarray(inputs["cos"], np.float32),
        np.asarray(inputs["sin"], np.float32),
        np.asarray(inputs["wq"], np.float32),
        np.asarray(inputs["wk"], np.float32),
        np.asarray(inputs["wv"], np.float32),
        np.asarray(inputs["wo"], np.float32),
    )
    res = run_bass_kernel_spmd(nc, in_maps, list(range(NCORES)), trace=trace)
    outs = [np.asarray(r["out"]).astype(np.float32) for r in res.results]
    y = np.stack([outs[TP * b] + outs[TP * b + 1] for b in range(B)])
    return y, res


def kernel(**inputs):
    y, _ = run(inputs, trace=False)
    return y
